# revision 40
# baseline (speedup 1.0000x reference)
"""Multi-head self-attention on 8 Trainium2 NeuronCores.

Problem: B=4, S=2048, D=1024, H=16 heads (dk=64), torch-Linear style
projections (y = x @ W.T + b), softmax attention, output projection.

Sharding: 8 cores = 4 batches x 2 head-groups (8 heads each).  Each core
computes, for its (batch b, group g):
    QT = (Wq_g/(8*sqrt(dk))) @ x_b.T + bq_g/(8*sqrt(dk))  [512, S]
         (scores are produced pre-divided by 8; the exp stage multiplies
          back: ACT uses Exp scale=8, DVE evaluates p(u)^8 with p ~ e^u)
    KT = Wk_g @ x_b.T                              [512, S]  (bk dropped: it only
                                                    shifts scores uniformly per
                                                    query and cancels in softmax)
    V  = x_b @ Wv_g.T + bv_g                       [S, 512]  (keys on partitions)
    per head h: scoresT = K_h @ Q_h.T              [S(keys), S(queries)]
    expT = exp(scoresT)            (no max-subtraction: |scores| < ~3.5)
        - scalar engine (ACT): exp of 768 of each 1024-col block (scale=8)
        - vector engine (DVE): exp of the other 256 cols via two custom
          micro-coded ops: p = 1+u(c0+u(c1+u*c2)) then p^8 (rel err < 3e-3)
    outT_h = V_h.T @ expT          [64, S], plus a ones-row matmul giving the
                                   softmax denominators per query
    normalized via 1/denominator broadcast, then
    partialT = Wo_g @ outT_all     [1024, S] in bf16
Host sums the two group partials per batch, transposes, and adds bo.

Device dtypes: bf16 matmul operands, f32 PSUM/exp/normalization, bf16 out.
"""

import math

import numpy as np
import ml_dtypes

import concourse.bass as bass
import concourse.bacc as bacc_mod
import concourse.mybir as mybir
import concourse.tile as tile
from concourse.bass_utils import run_bass_kernel_spmd

BF16 = mybir.dt.bfloat16
F32 = mybir.dt.float32
AF = mybir.ActivationFunctionType

B, S, D, H = 4, 2048, 1024, 16
DK = D // H  # 64
NCORES = 8
GROUPS = 2  # tensor-parallel head groups
DG = D // GROUPS  # 512 features per group
P = 128
FT = DG // P  # 4 feature tiles per group == head pairs

# exp split: per [128, 1024] score block, ACT does cols [0:ACOLS] with
# Exp(scale=SC); DVE does [ACOLS:1024] in ONE custom op: p(u)^4 with
# p = c0 + u*(c1 + u*(c2 + u*c3)) ~ e^u (c3 rides in via the Src1 spill).
SC = 4.0
ACOLS = 624
# rel-minimax deg-3 fitted on u in [-0.875, 0.875] (score in [-3.5, 3.5]);
# p^4 rel err <= 1.2e-2 there.  A DVE-handled query gets the polynomial for
# ALL its keys, so the common bias largely cancels in its own softmax;
# measured end-to-end contribution ~0.5%.
EXP_C0 = 0.99773437
EXP_C1 = 1.0064234
EXP_C2 = 0.5302388
EXP_C3 = 0.16039258

_DVE_OPS = {}


def _register_dve_exp_ops():
    """Register the two custom DVE micro-ops used for the vector-engine
    exp path. Idempotent; appends to concourse's module-level op registry
    (sha computed at registration, same as DveOp.compile would)."""
    if _DVE_OPS:
        return _DVE_OPS
    from concourse.dve_spec import (
        Spec, Src0, C0, C1, C2, C3, sq, lower, _has_src1, _spill_c3_to_src1,
    )
    from concourse.dve_uop import DveOpSpec
    from concourse.dve_ops import (
        DveOp,
        OPS,
        CUSTOM_DVE_SPECS,
        _SUB_OPCODE_FOR_NAME,
        _CUSTOM_DVE_ROW_BASE,
    )

    u = Src0

    def _ref_exp4p(in0, in1, s0, s1, imm2):
        import numpy as np

        p = s0 + in0 * (s1 + in0 * (imm2 + in0 * in1))
        return (p * p) ** 2

    specs = {
        "ANT_EXP4P": Spec(
            body=_spill_c3_to_src1(sq(sq(C0 + u * (C1 + u * (C2 + u * C3))))),
            reference=_ref_exp4p,
        ),
    }
    for name, sp in specs.items():
        if name not in _SUB_OPCODE_FOR_NAME:
            row = _CUSTOM_DVE_ROW_BASE + len(OPS)
            op = DveOp(name, sp, subdim=False, uops_sha={})
            _SUB_OPCODE_FOR_NAME[name] = row
            OPS.append(op)
            CUSTOM_DVE_SPECS[name] = sp
            for ver in ("v3", "v4"):
                s = DveOpSpec(
                    name=name, opcode=row, uops=lower(sp, ver=ver),
                    rd1_en=_has_src1(sp),
                )
                op.uops_sha[ver] = s.sha(ver)
        else:
            op = next(o for o in OPS if o.name == name)
        _DVE_OPS[name] = op
    return _DVE_OPS


def build_attention_nc(seq: int = S) -> bass.Bass:
    KB = seq // P  # key blocks
    DKB = D // P  # 8 contraction blocks for projections
    QH = min(512, seq)  # query stripe processed per attention pass
    NQH = seq // QH
    QC = min(512, QH)  # matmul moving-operand chunk
    NQC = seq // QC  # chunks per full seq
    DT = D // P
    XC = 512  # x DMA column-chunk width
    NXC = seq // XC

    ops = _register_dve_exp_ops()
    exp4p = ops["ANT_EXP4P"]

    nc = bacc_mod.Bacc("TRN2", num_devices=NCORES)
    xt_d = nc.declare_dram_parameter("xt", [D, seq], BF16, isOutput=False)
    wqt_d = nc.declare_dram_parameter("wqt", [D, DG], BF16, isOutput=False)
    wkt_d = nc.declare_dram_parameter("wkt", [D, DG], BF16, isOutput=False)
    wvt_d = nc.declare_dram_parameter("wvt", [D, DG], BF16, isOutput=False)
    wot_d = nc.declare_dram_parameter("wot", [DG, D], BF16, isOutput=False)
    bq_d = nc.declare_dram_parameter("bqs", [P, FT], F32, isOutput=False)
    out_d = nc.declare_dram_parameter("out", [D, seq], BF16, isOutput=True)

    with tile.TileContext(nc) as tc:
        with tc.tile_pool(name="persist", bufs=1) as persist:
            bq_sb = persist.tile([P, FT], F32, name="bq_sb")
            nc.sync.dma_start(bq_sb, bq_d[:, :])
            c3_sb = persist.tile([P, 1], F32, name="c3_sb")
            nc.vector.memset(c3_sb, EXP_C3)

            qt_sb = [persist.tile([P, seq], BF16, name=f"qt{i}") for i in range(FT)]
            kt_sb = [persist.tile([P, seq], BF16, name=f"kt{i}") for i in range(FT)]
            # v2 holds, per 128-col head block: even heads [V_h | ones],
            # odd heads [ones | V_h] — the ones columns make the PV matmul
            # also produce the softmax denominators on the other 64 rows.
            v2_sb = [persist.tile([P, 2 * DG], BF16, name=f"v{i}") for i in range(KB)]
            wot_sb = [persist.tile([P, D], BF16, name=f"wot{i}") for i in range(FT)]
            onorm = [persist.tile([P, seq], BF16, name=f"onorm{i}") for i in range(FT)]

            # memset the ones-pattern of v2 now: overlaps the input DMA wait
            for kb in range(KB):
                nc.vector.memset(v2_sb[kb], 1.0)

            # ---------------- phase 1: projections ----------------
            with (
                tc.tile_pool(name="xw", bufs=1) as xw_pool,
                tc.tile_pool(name="pps", bufs=4, space="PSUM") as proj_ps,
            ):
                # Slab tiles: each holds all 8 contraction sub-tiles side by
                # side so ONE dma_start (whose packets spread over all 16 DMA
                # engines) loads a whole slab — descriptor issue costs ~650ns
                # each and would otherwise serialize the startup.
                xt_all = xw_pool.tile([P, DKB * seq], BF16, name="xts")
                wq_all = xw_pool.tile([P, DKB * DG], BF16, name="wqts")
                wk_all = xw_pool.tile([P, DKB * DG], BF16, name="wkts")
                wv_all = xw_pool.tile([P, DKB * DG], BF16, name="wvts")
                xt_sb = [xt_all[:, i * seq : (i + 1) * seq] for i in range(DKB)]
                wqt_sb = [wq_all[:, i * DG : (i + 1) * DG] for i in range(DKB)]
                wkt_sb = [wk_all[:, i * DG : (i + 1) * DG] for i in range(DKB)]
                wvt_sb = [wv_all[:, i * DG : (i + 1) * DG] for i in range(DKB)]

                def wdma(dst_all, w_d):
                    return (
                        dst_all.rearrange("p (k g) -> p k g", g=DG),
                        w_d.rearrange("(k p) g -> p k g", p=P),
                    )

                def xsrc(c):
                    csl = slice(c * XC, (c + 1) * XC)
                    return (
                        xt_all.rearrange("p (k s) -> p k s", s=seq)[:, :, csl],
                        xt_d.rearrange("(k p) s -> p k s", p=P)[:, :, csl],
                    )

                nc.sync.dma_start(*xsrc(0))
                nc.scalar.dma_start(*wdma(wq_all, wqt_d))
                nc.scalar.dma_start(*wdma(wk_all, wkt_d))
                nc.sync.dma_start(*xsrc(1))
                nc.scalar.dma_start(*xsrc(2))
                nc.sync.dma_start(*xsrc(3))
                nc.scalar.dma_start(*wdma(wv_all, wvt_d))
                for ft in range(FT):
                    nc.sync.dma_start(wot_sb[ft], wot_d[ft * P : (ft + 1) * P, :])

                # QT / KT: features on partitions, queries on free dim.
                # All four QT blocks before the KT blocks per column chunk:
                # the k weights stream in on the sync queue behind x chunk 0,
                # so KT starting ~8us into the phase never stalls on them.
                for c in range(NQC):
                    csl = slice(c * QC, (c + 1) * QC)
                    for ft in range(FT):
                        fsl = slice(ft * P, (ft + 1) * P)
                        psq = proj_ps.tile([P, QC], F32, name="psq", tag="proj")
                        for k in range(DKB):
                            nc.tensor.matmul(
                                psq,
                                lhsT=wqt_sb[k][:, fsl],
                                rhs=xt_sb[k][:, csl],
                                start=k == 0,
                                stop=k == DKB - 1,
                            )
                        nc.scalar.activation(
                            qt_sb[ft][:, csl], psq, AF.Identity,
                            bias=bq_sb[:, ft : ft + 1],
                        )
                    for ft in range(FT):
                        fsl = slice(ft * P, (ft + 1) * P)
                        psk = proj_ps.tile([P, QC], F32, name="psk", tag="proj")
                        for k in range(DKB):
                            nc.tensor.matmul(
                                psk,
                                lhsT=wkt_sb[k][:, fsl],
                                rhs=xt_sb[k][:, csl],
                                start=k == 0,
                                stop=k == DKB - 1,
                            )
                        nc.vector.tensor_copy(kt_sb[ft][:, csl], psk)

                # V: keys on partitions, features on free dim.  bv is folded
                # into the host-side bias (attention weights sum to 1, so
                # out += Wo @ bv exactly).  The strided copy into v2 runs on
                # the scalar engine (idle here) so the vector engine never
                # limits this section.
                for kb in range(KB):
                    ksl = slice(kb * P, (kb + 1) * P)
                    psv = proj_ps.tile([P, DG], F32, name="psv", tag="proj")
                    for k in range(DKB):
                        nc.tensor.matmul(
                            psv,
                            lhsT=xt_sb[k][:, ksl],
                            rhs=wvt_sb[k],
                            start=k == 0,
                            stop=k == DKB - 1,
                        )
                    # even heads -> cols [256q+0:64); odd heads -> [256q+192:256)
                    nc.scalar.activation(
                        v2_sb[kb].rearrange("p (q t c) -> p q t c", t=4, c=64)[
                            :, :, 0::3, :
                        ],
                        psv.rearrange("p (q t c) -> p q t c", t=2, c=64),
                        AF.Copy,
                    )

            # ---------------- phase 2: attention ----------------
            with (
                tc.tile_pool(name="sps", bufs=2, space="PSUM") as s_ps,
                tc.tile_pool(name="pvps", bufs=4, space="PSUM") as pv_ps,
                tc.tile_pool(name="epool", bufs=5) as e_pool,
                tc.tile_pool(name="mpool", bufs=2) as m_pool,
            ):
                # Software-pipelined attention: scores/exp for key-block kb
                # are emitted at tick t, the PV matmuls that consume them at
                # tick t+DEPTH — continuously ACROSS stripe boundaries, so the
                # ~1.5us scores->exp->pv chain never drains/refills per
                # stripe.  Each stripe's normalization chain is scheduled a
                # few ticks after its final PV accumulation pops, spread out
                # so no engine queue absorbs more than one ~0.7us blip.
                DEPTH = 3
                epipe = []  # (pv0, pv1, h0c, h1c, kp, e0, e1, norm or None)
                pend = []  # (due_tick, closure)
                tick = 0

                def emit_pv(entry):
                    pv0, pv1, h0c, h1c, kp, e0, e1, norm = entry
                    nc.tensor.matmul(
                        pv0, lhsT=v2_sb[kp][:, h0c], rhs=e0,
                        start=kp == 0, stop=kp == KB - 1,
                    )
                    nc.tensor.matmul(
                        pv1, lhsT=v2_sb[kp][:, h1c], rhs=e1,
                        start=kp == 0, stop=kp == KB - 1,
                    )
                    if norm is not None:
                        norm()

                def flush_due(now):
                    while pend and pend[0][0] <= now:
                        pend.pop(0)[1]()

                for qh in range(NQH):
                    qsl = slice(qh * QH, (qh + 1) * QH)
                    for pr in range(FT):  # head pair == feature tile
                        h0c = slice((2 * pr) * P, (2 * pr + 1) * P)
                        h1c = slice((2 * pr + 1) * P, (2 * pr + 2) * P)
                        pv0 = pv_ps.tile([P, QH], F32, name="pv0", tag="pv")
                        pv1 = pv_ps.tile([P, QH], F32, name="pv1", tag="pv")

                        def make_norm(pr=pr, qsl=qsl, pv0=pv0, pv1=pv1,
                                      last=False):
                            # pv0 rows 0:64 = outT_h0, rows 64:128 = den_h0
                            # pv1 rows 0:64 = den_h1,  rows 64:128 = outT_h1
                            rsw = m_pool.tile([P, QH], F32, name="rsw",
                                              tag="rsw")

                            def norm_a():
                                # den staging on the scalar engine (it has
                                # slack), reciprocal on vector, then DMA-swap
                                # the reciprocal halves across partitions.
                                den = m_pool.tile([P, QH], F32, name="den",
                                                  tag="den")
                                nc.scalar.activation(
                                    den[0:64, :], pv1[0:64, :], AF.Copy
                                )
                                if last:
                                    # end of phase 2: split the two copies
                                    # across engines to shorten the serial
                                    # tail before the phase-3 pool barrier.
                                    nc.vector.tensor_copy(
                                        den[64:128, :], pv0[64:128, :]
                                    )
                                else:
                                    nc.scalar.activation(
                                        den[64:128, :], pv0[64:128, :], AF.Copy
                                    )
                                rcu = m_pool.tile([P, QH], F32, name="rcu",
                                                  tag="rcu")
                                nc.vector.reciprocal_approx_fast(rcu, den)
                                hq = QH // 2
                                nc.sync.dma_start(
                                    rsw[0:64, 0:hq], rcu[64:128, 0:hq]
                                )
                                nc.sync.dma_start(
                                    rsw[0:64, hq:QH], rcu[64:128, hq:QH]
                                )
                                nc.sync.dma_start(
                                    rsw[64:128, 0:hq], rcu[0:64, 0:hq]
                                )
                                nc.sync.dma_start(
                                    rsw[64:128, hq:QH], rcu[0:64, hq:QH]
                                )

                            def norm_b0():
                                nc.vector.tensor_tensor(
                                    onorm[pr][0:64, qsl], pv0[0:64, :],
                                    rsw[0:64, :], mybir.AluOpType.mult,
                                )

                            def norm_b1():
                                nc.vector.tensor_tensor(
                                    onorm[pr][64:128, qsl], pv1[64:128, :],
                                    rsw[64:128, :], mybir.AluOpType.mult,
                                )

                            def schedule(now):
                                pend.append((now + 2, norm_a))
                                pend.append((now + 6, norm_b0))
                                pend.append((now + 10, norm_b1))
                                pend.sort(key=lambda e: e[0])

                            if last:
                                def eager():
                                    norm_a()
                                    norm_b0()
                                    norm_b1()
                                return eager
                            return schedule

                        is_last = qh == NQH - 1 and pr == FT - 1
                        schedule_norm = make_norm(last=is_last)

                        for kb in range(KB):
                            ksl = slice(kb * P, (kb + 1) * P)
                            s0 = s_ps.tile([P, QH], F32, name="s0", tag="s0")
                            s1 = s_ps.tile([P, QH], F32, name="s1", tag="s1")
                            nc.tensor.matmul(
                                s0,
                                lhsT=kt_sb[pr][0:64, ksl],
                                rhs=qt_sb[pr][0:64, qsl],
                                start=True, stop=True,
                            )
                            nc.tensor.matmul(
                                s1,
                                lhsT=kt_sb[pr][64:128, ksl],
                                rhs=qt_sb[pr][64:128, qsl],
                                start=True, stop=True,
                            )
                            e0 = e_pool.tile([P, QH], BF16, name="e0", tag="e0")
                            e1 = e_pool.tile([P, QH], BF16, name="e1", tag="e1")
                            nc.scalar.activation(e0, s0, AF.Exp, scale=SC)
                            nc.vector._custom_dve(
                                exp4p, out=e1, in0=s1, in1=c3_sb[:, 0:1],
                                s0=EXP_C0, s1=EXP_C1, imm2=EXP_C2,
                            )
                            epipe.append(
                                (pv0, pv1, h0c, h1c, kb, e0, e1,
                                 (lambda now=tick, f=schedule_norm,
                                  L=is_last: (f() if L else f(now)))
                                 if kb == KB - 1 else None)
                            )
                            if len(epipe) > DEPTH:
                                emit_pv(epipe.pop(0))
                            flush_due(tick)
                            tick += 1
                while epipe:
                    emit_pv(epipe.pop(0))
                    tick += 1
                while pend:
                    pend.pop(0)[1]()

            # ---------------- phase 3: output projection ----------------
            # c outer so the first output columns only need the early query
            # stripes; bf16 partials DMA'd straight from PSUM.
            with tc.tile_pool(name="ops", bufs=6, space="PSUM") as o_ps:
                with tc.tile_pool(name="osb", bufs=4) as o_sb_pool:
                    for c in range(NQC):
                        csl = slice(c * QC, (c + 1) * QC)
                        for dt in range(DT):
                            dsl = slice(dt * P, (dt + 1) * P)
                            pso = o_ps.tile([P, QC], F32, name="pso", tag="po")
                            for ft in range(FT):
                                nc.tensor.matmul(
                                    pso,
                                    lhsT=wot_sb[ft][:, dsl],
                                    rhs=onorm[ft][:, csl],
                                    start=ft == 0,
                                    stop=ft == FT - 1,
                                )
                            o_sb = o_sb_pool.tile([P, QC], BF16, name="o_sb",
                                                  tag="osb")
                            nc.vector.tensor_copy(o_sb, pso)
                            nc.sync.dma_start(out_d[dsl, csl], o_sb)

    return nc


_CACHE: dict = {}


def _get_nc(seq: int = S) -> bass.Bass:
    key = f"nc{seq}"
    if key not in _CACHE:
        nc = build_attention_nc(seq)
        nc.finalize()  # runs Bacc.compile(): reg alloc + wait legalization
        _CACHE[key] = nc
    return _CACHE[key]


def make_in_maps(x, Wq, bq, Wk, Wv, bv, Wo, seq: int = S):
    bf = ml_dtypes.bfloat16
    scale = 1.0 / (SC * math.sqrt(DK))
    x = np.asarray(x, np.float32)
    Wq = np.asarray(Wq, np.float32)
    bq = np.asarray(bq, np.float32)
    Wk = np.asarray(Wk, np.float32)
    Wv = np.asarray(Wv, np.float32)
    bv = np.asarray(bv, np.float32)
    Wo = np.asarray(Wo, np.float32)
    in_maps = []
    for core in range(NCORES):
        b, g = divmod(core, GROUPS)
        gsl = slice(g * DG, (g + 1) * DG)
        in_maps.append(
            {
                "xt": np.ascontiguousarray(x[b, :seq, :].T).astype(bf),
                "wqt": np.ascontiguousarray((Wq[gsl, :] * scale).T).astype(bf),
                "wkt": np.ascontiguousarray(Wk[gsl, :].T).astype(bf),
                "wvt": np.ascontiguousarray(Wv[gsl, :].T).astype(bf),
                "wot": np.ascontiguousarray(Wo[:, gsl].T).astype(bf),
                "bqs": np.ascontiguousarray(
                    (bq[gsl] * scale).astype(np.float32).reshape(FT, P).T
                ),
            }
        )
    return in_maps


def run_device(in_maps, seq: int = S, trace: bool = False):
    nc = _get_nc(seq)
    return run_bass_kernel_spmd(nc, in_maps, list(range(NCORES)), trace=trace)


def kernel(x, Wq, bq, Wk, bk, Wv, bv, Wo, bo):
    in_maps = make_in_maps(x, Wq, bq, Wk, Wv, bv, Wo)
    res = run_device(in_maps).results
    # bv passes through the attention average unchanged (weights sum to 1),
    # so its contribution to the output is exactly Wo @ bv, added here.
    bias = np.asarray(bo, np.float32) + np.asarray(Wo, np.float32) @ np.asarray(
        bv, np.float32
    )
    out = np.empty((B, S, D), np.float32)
    for b in range(B):
        acc = res[2 * b]["out"].astype(np.float32) + res[2 * b + 1]["out"].astype(
            np.float32
        )
        out[b] = acc.T + bias[None, :]
    return out
